# revision 1
# baseline (speedup 1.0000x reference)
"""DCM (dynamic conv module) Trainium2 kernel.

Reference computation (per sample b, channel c):
  f[b,c,3,3]  = adaptive_avg_pool2d(x[b,c], 3)        # dynamic depthwise filter
  out[b,c]    = depthwise_conv3x3(x[b,c], f[b,c])     # zero padding 1
  y           = relu(batchnorm_train(out, gamma, beta))  # batch stats over (B,H,W)

Sharding: data-parallel over batch B=16 across 8 cores (2 samples/core).
Sync-BN via a [C,2] AllReduce of per-channel (sum, sumsq).

Per-core layout: channels C=128 on partitions, free dim = H*W per sample.
Conv = 9 shifted taps done as diag(f_tap) matmuls (float32r, 1 cyc/row)
accumulated in PSUM; horizontal zero-padding handled by letting taps wrap
across row ends and subtracting the wrapped term on the two edge columns
(DVE scalar_tensor_tensor fixups). Conv output stays resident in SBUF
(128 KiB/partition) until the stats AllReduce, then BN+ReLU is applied
in-place (ACT/DVE split) and DMA'd out.
"""

import os
import numpy as np

# ---------------------------------------------------------------- constants
B, C, H, W = 16, 128, 128, 128
N_CORES = 8
BL = B // N_CORES          # samples per core
HW = H * W                 # 16384 free elems per plane
FS = 3
BN_EPS = 1e-5

ROWS = 16                  # output rows per psum tile
NCHUNK = H // ROWS         # 8 chunks per plane
TILE_F = ROWS * W          # 2048 free elems per psum tile
XT_F = (ROWS + 2) * W + 2  # x chunk with halo rows + 1 elem pad each end
NPSUM = NCHUNK * BL        # psum tiles per core

# adaptive_avg_pool2d(3) bin boundaries (PyTorch convention)
SH = [(i * H) // FS for i in range(FS)]
EH = [-((-(i + 1) * H) // FS) for i in range(FS)]
SW = [(i * W) // FS for i in range(FS)]
EW = [-((-(i + 1) * W) // FS) for i in range(FS)]

TAPS = [(di, dj) for di in (-1, 0, 1) for dj in (-1, 0, 1)]  # t = 3*(di+1)+(dj+1)

MM_N = 512                 # fp32 moving-operand max per matmul
NSL = TILE_F // MM_N       # bank slices per psum tile

# Matmul operand dtype for the conv taps. The PE runs fp32 at 4 cycles/row;
# float32r (rounded fp32) and bf16 run at 1 cycle/row. float32r operands must
# be produced as float32r per the BIR verifier, so the tap pass streams x a
# second time from a dedicated DRAM tensor declared at this dtype.
MM_DTYPE = os.environ.get("DCM_MM_DTYPE", "f32r")


def _counts_recip():
    cr = np.empty((C, FS * FS), dtype=np.float32)
    for i in range(FS):
        for j in range(FS):
            cr[:, 3 * i + j] = 1.0 / float((EH[i] - SH[i]) * (EW[j] - SW[j]))
    return cr


def build_nc(n_cores: int = N_CORES):
    """Build + compile the per-core Bass program (identical on all cores)."""
    import concourse.bacc as bacc
    import concourse.tile as tile
    from concourse import mybir

    f32 = mybir.dt.float32
    f32r = mybir.dt.float32r
    AT = mybir.ActivationFunctionType
    OP = mybir.AluOpType
    AX = mybir.AxisListType

    ntot = float(n_cores * BL * HW)   # BN element count per channel

    nc = bacc.Bacc(
        "TRN2",
        target_bir_lowering=False,
        debug=False,
        num_devices=n_cores,
    )

    if MM_DTYPE == "f32r":
        mdt = mybir.dt.float32r
    elif MM_DTYPE == "bf16":
        mdt = mybir.dt.bfloat16
    else:
        mdt = f32

    x_d = nc.dram_tensor("x", [BL, C, HW], f32, kind="ExternalInput").ap()
    x2_d = (
        nc.dram_tensor("x2", [BL, C, HW], mdt, kind="ExternalInput").ap()
        if MM_DTYPE != "f32"
        else x_d
    )
    gamma_d = nc.dram_tensor("gamma", [C, 1], f32, kind="ExternalInput").ap()
    beta_d = nc.dram_tensor("beta", [C, 1], f32, kind="ExternalInput").ap()
    ident_d = nc.dram_tensor("ident", [C, C], f32, kind="ExternalInput").ap()
    crecip_d = nc.dram_tensor("crecip", [C, FS * FS], f32, kind="ExternalInput").ap()
    y_d = nc.dram_tensor("y", [BL, C, HW], f32, kind="ExternalOutput").ap()

    with tile.TileContext(nc) as tc:
        with (
            tc.tile_pool(name="singles", bufs=1) as singles,
            tc.tile_pool(name="xpool", bufs=2) as xpool,
            tc.tile_pool(name="outres", bufs=NPSUM) as outres,
            tc.tile_pool(name="psum", bufs=2, space="PSUM") as psum,
            tc.tile_pool(name="colsp", bufs=2) as colsp,
            tc.tile_pool(name="fpool", bufs=2) as fpool,
            tc.tile_pool(name="diagp", bufs=2 * FS * FS) as diagp,
            tc.tile_pool(name="statp", bufs=1) as statp,
            tc.tile_pool(name="dram", bufs=1, space="DRAM") as dram,
        ):
            # ---- constants
            gamma_s = singles.tile([C, 1], f32, tag="gamma")
            nc.sync.dma_start(out=gamma_s[:], in_=gamma_d[:, :])
            beta_s = singles.tile([C, 1], f32, tag="beta")
            nc.sync.dma_start(out=beta_s[:], in_=beta_d[:, :])
            ident_s = singles.tile([C, C], f32, tag="ident")
            nc.sync.dma_start(out=ident_s[:], in_=ident_d[:, :])
            crecip_s = singles.tile([C, FS * FS], f32, tag="crecip")
            nc.sync.dma_start(out=crecip_s[:], in_=crecip_d[:, :])

            sums = statp.tile([C, NPSUM], f32, tag="sums")
            sumsq = statp.tile([C, NPSUM], f32, tag="sumsq")

            # Dummy warm-up AllReduce issued at kernel start: absorbs the
            # one-time ncfw ramp so the real stats AllReduce on the critical
            # path is cheaper. Runs concurrently with the pooling pass.
            warm = statp.tile([C, 2], f32, tag="warm")
            nc.gpsimd.memset(warm[:], 0.0)
            dw_in = dram.tile([C, 2], f32, tag="dw_in")
            dw_out = dram.tile([C, 2], f32, tag="dw_out")
            nc.sync.dma_start(out=dw_in[:], in_=warm[:])
            nc.gpsimd.collective_compute(
                "AllReduce",
                OP.add,
                replica_groups=[list(range(n_cores))],
                ins=[dw_in[:].opt()],
                outs=[dw_out[:].opt()],
            )

            out_tiles = []
            kpt = 0  # global psum-tile index

            def load_chunk(s, c, src=x_d, dtype=f32, tag="xt"):
                """DMA one halo chunk of plane s into a fresh x tile."""
                xt = xpool.tile([C, XT_F], dtype, tag=tag)
                # float32r has no memset encoding; same bits as f32
                mview = xt[:].bitcast(f32) if dtype == mybir.dt.float32r else xt[:]
                r_lo = c * ROWS - 1
                r_hi = c * ROWS + ROWS + 1
                # 1-elem pads at both ends (read by corner-wrap taps; must be
                # finite so the fixup subtraction cancels exactly). On DVE —
                # gpsimd can be blocked for long stretches by the collective.
                nc.vector.memset(mview[:, 0:1], 0.0)
                nc.vector.memset(mview[:, XT_F - 1:XT_F], 0.0)
                if r_lo < 0:
                    nc.vector.memset(mview[:, 1:1 + W], 0.0)
                if r_hi > H:
                    nc.vector.memset(mview[:, 1 + (ROWS + 1) * W:1 + (ROWS + 2) * W], 0.0)
                src_lo = max(r_lo, 0) * W
                src_hi = min(r_hi, H) * W
                dst_lo = 1 + (max(r_lo, 0) - r_lo) * W
                nc.sync.dma_start(
                    out=xt[:, dst_lo:dst_lo + (src_hi - src_lo)],
                    in_=src[s, :, src_lo:src_hi],
                )
                return xt

            for s in range(BL):
                # ---------------- phase 1: pooling pass over plane s
                # (no halo needed; plain 16-row tiles, triple buffered so the
                # DMA cadence, not the pool slots, paces the pipeline)
                colS = colsp.tile([C, FS, H], f32, tag="colS")
                for c in range(NCHUNK):
                    xt = xpool.tile([C, TILE_F], f32, tag="xt", bufs=3)
                    nc.sync.dma_start(
                        out=xt[:], in_=x_d[s, :, c * TILE_F:(c + 1) * TILE_F]
                    )
                    xv = xt[:].rearrange("p (r w) -> p r w", w=W)
                    for j in range(FS):
                        nc.vector.tensor_reduce(
                            out=colS[:, j, c * ROWS:(c + 1) * ROWS],
                            in_=xv[:, :, SW[j]:EW[j]],
                            axis=AX.X,
                            op=OP.add,
                        )

                # ---------------- filter f [C,9] and diag weights
                fT = fpool.tile([C, FS * FS], f32, tag="fT")
                for i in range(FS):
                    for j in range(FS):
                        k = 3 * i + j
                        nc.vector.tensor_reduce(
                            out=fT[:, k:k + 1],
                            in_=colS[:, j, SH[i]:EH[i]],
                            axis=AX.X,
                            op=OP.add,
                        )
                nc.vector.tensor_mul(fT[:], fT[:], crecip_s[:])
                # fixup scalars at the matmul operand precision so the
                # subtraction matches what the PE added
                fneg = fpool.tile(
                    [C, FS * FS], mdt if MM_DTYPE == "bf16" else f32, tag="fneg"
                )
                nc.vector.tensor_scalar_mul(fneg[:], fT[:], -1.0)
                diags = []
                for t in range(FS * FS):
                    dg = diagp.tile([C, C], mdt, tag="diag")
                    nc.vector.tensor_scalar_mul(dg[:], ident_s[:], fT[:, t:t + 1])
                    diags.append(dg)

                # ---------------- phase 2: conv taps (x streamed a 2nd time)
                for c in range(NCHUNK):
                    xt = load_chunk(s, c, src=x2_d, dtype=mdt, tag="xt2")
                    # DVE fixups read the same tile; f32r is bit-identical f32
                    xtv = xt[:].bitcast(f32) if MM_DTYPE == "f32r" else xt[:]
                    pt = psum.tile([C, TILE_F], f32, tag="pt")
                    for sl in range(NSL):
                        for t, (di, dj) in enumerate(TAPS):
                            base = 1 + (di + 1) * W + dj + sl * MM_N
                            nc.tensor.matmul(
                                pt[:, sl * MM_N:(sl + 1) * MM_N],
                                diags[t][:],
                                xt[:, base:base + MM_N],
                                start=(t == 0),
                                stop=(t == FS * FS - 1),
                            )
                    # edge-column fixups: subtract the horizontally wrapped term
                    pv = pt[:].rearrange("p (r w) -> p r w", w=W)
                    for i, di in enumerate((-1, 0, 1)):
                        # w = 0 read x[h+di, -1] -> wrapped to (h+di-1, W-1)
                        src = xtv[:, (di + 1) * W:(di + 1) * W + ROWS * W].rearrange(
                            "p (r w) -> p r w", w=W
                        )[:, :, 0:1]
                        nc.vector.scalar_tensor_tensor(
                            out=pv[:, :, 0:1],
                            in0=src,
                            scalar=fneg[:, 3 * i:3 * i + 1],
                            in1=pv[:, :, 0:1],
                            op0=OP.mult,
                            op1=OP.add,
                        )
                        # w = W-1 read x[h+di, W] -> wrapped to (h+di+1, 0),
                        # i.e. flat cells 1+(di+2)*W + r*W; expressed as col
                        # W-1 of a view starting 127 elems earlier
                        s0 = (di + 1) * W + 2
                        src = xtv[:, s0:s0 + ROWS * W].rearrange(
                            "p (r w) -> p r w", w=W
                        )[:, :, W - 1:W]
                        nc.vector.scalar_tensor_tensor(
                            out=pv[:, :, W - 1:W],
                            in0=src,
                            scalar=fneg[:, 3 * i + 2:3 * i + 3],
                            in1=pv[:, :, W - 1:W],
                            op0=OP.mult,
                            op1=OP.add,
                        )
                    # PSUM -> resident SBUF copy, fused per-channel sum
                    ot = outres.tile([C, TILE_F], f32, tag="ot")
                    nc.scalar.activation(
                        out=ot[:], in_=pt[:], func=AT.Copy,
                        accum_out=sums[:, kpt:kpt + 1],
                    )
                    # sum of squares; squared values overwrite the psum tile
                    # in place (only accum_out is kept)
                    nc.scalar.activation(
                        out=pt[:], in_=pt[:], func=AT.Square,
                        accum_out=sumsq[:, kpt:kpt + 1],
                    )
                    out_tiles.append((s, c, ot))
                    kpt += 1

            # ---------------- sync-BN stats AllReduce
            arin = statp.tile([C, 2], f32, tag="arin")
            nc.vector.tensor_reduce(out=arin[:, 0:1], in_=sums[:], axis=AX.X, op=OP.add)
            nc.vector.tensor_reduce(out=arin[:, 1:2], in_=sumsq[:], axis=AX.X, op=OP.add)
            d_in = dram.tile([C, 2], f32, tag="d_in")
            d_out = dram.tile([C, 2], f32, tag="d_out")
            nc.sync.dma_start(out=d_in[:], in_=arin[:])
            nc.gpsimd.collective_compute(
                "AllReduce",
                OP.add,
                replica_groups=[list(range(n_cores))],
                ins=[d_in[:].opt()],
                outs=[d_out[:].opt()],
            )
            aro = statp.tile([C, 2], f32, tag="aro")
            nc.sync.dma_start(out=aro[:], in_=d_out[:])

            # ---------------- BN scale/shift (all [C,1], fp32)
            mean = statp.tile([C, 1], f32, tag="mean")
            nc.vector.tensor_scalar_mul(mean[:], aro[:, 0:1], 1.0 / ntot)
            ex2 = statp.tile([C, 1], f32, tag="ex2")
            nc.vector.tensor_scalar_mul(ex2[:], aro[:, 1:2], 1.0 / ntot)
            var = statp.tile([C, 1], f32, tag="var")
            nc.vector.tensor_mul(var[:], mean[:], mean[:])
            nc.vector.tensor_sub(var[:], ex2[:], var[:])
            veps = statp.tile([C, 1], f32, tag="veps")
            nc.vector.tensor_scalar_add(veps[:], var[:], BN_EPS)
            eps_t = statp.tile([C, 1], f32, tag="eps_t")
            nc.vector.memset(eps_t[:], BN_EPS)
            sd = statp.tile([C, 1], f32, tag="sd")
            nc.scalar.activation(out=sd[:], in_=var[:], func=AT.Sqrt, bias=eps_t[:])
            z = statp.tile([C, 1], f32, tag="z")
            nc.vector.reciprocal(z[:], sd[:])
            # one Newton step: z <- z * (1.5 - 0.5 * veps * z^2)
            nt = statp.tile([C, 1], f32, tag="nt")
            nc.vector.tensor_mul(nt[:], z[:], z[:])
            nc.vector.tensor_mul(nt[:], nt[:], veps[:])
            nc.vector.tensor_scalar(
                out=nt[:], in0=nt[:], scalar1=-0.5, scalar2=1.5,
                op0=OP.mult, op1=OP.add,
            )
            nc.vector.tensor_mul(z[:], z[:], nt[:])
            scale_t = statp.tile([C, 1], f32, tag="scale_t")
            nc.vector.tensor_mul(scale_t[:], gamma_s[:], z[:])
            shift_t = statp.tile([C, 1], f32, tag="shift_t")
            nc.vector.tensor_mul(shift_t[:], mean[:], scale_t[:])
            nc.vector.tensor_sub(shift_t[:], beta_s[:], shift_t[:])

            # ---------------- BN apply + ReLU + writeback (ACT / DVE split;
            # DVE needs 2 ops per tile vs ACT's 1, so ACT takes ~10/16)
            for idx, (s, c, ot) in enumerate(out_tiles):
                if idx % 8 < 5:
                    nc.scalar.activation(
                        out=ot[:], in_=ot[:], func=AT.Relu,
                        scale=scale_t[:], bias=shift_t[:],
                    )
                else:
                    nc.vector.tensor_scalar(
                        out=ot[:], in0=ot[:],
                        scalar1=scale_t[:], scalar2=shift_t[:],
                        op0=OP.mult, op1=OP.add,
                    )
                    nc.vector.tensor_scalar_max(ot[:], ot[:], 0.0)
                nc.sync.dma_start(
                    out=y_d[s, :, c * TILE_F:(c + 1) * TILE_F], in_=ot[:],
                )

    nc.compile()
    return nc


_NC_CACHE = {}


def _get_nc(n_cores: int = N_CORES):
    if n_cores not in _NC_CACHE:
        _NC_CACHE[n_cores] = build_nc(n_cores)
    return _NC_CACHE[n_cores]


def make_in_maps(x: np.ndarray, gamma: np.ndarray, beta: np.ndarray,
                 n_cores: int = N_CORES):
    x_r = np.ascontiguousarray(
        np.asarray(x, dtype=np.float32).reshape(B, C, HW)
    )
    g = np.ascontiguousarray(np.asarray(gamma, dtype=np.float32).reshape(C, 1))
    b = np.ascontiguousarray(np.asarray(beta, dtype=np.float32).reshape(C, 1))
    ident = np.eye(C, dtype=np.float32)
    crecip = _counts_recip()
    if MM_DTYPE == "bf16":
        import ml_dtypes

        x2 = x_r.astype(ml_dtypes.bfloat16)
    elif MM_DTYPE == "f32r":
        x2 = x_r  # float32r is bit-identical to float32
    else:
        x2 = None
    maps = []
    for core in range(n_cores):
        m = {
            "x": x_r[core * BL:(core + 1) * BL],
            "gamma": g,
            "beta": b,
            "ident": ident,
            "crecip": crecip,
        }
        if x2 is not None:
            m["x2"] = x2[core * BL:(core + 1) * BL]
        maps.append(m)
    return maps


def kernel(x, gamma, beta):
    from concourse import bass_utils

    nc = _get_nc(N_CORES)
    in_maps = make_in_maps(x, gamma, beta, N_CORES)
    res = bass_utils.run_bass_kernel_spmd(nc, in_maps, core_ids=list(range(N_CORES)))
    y = np.concatenate([res.results[c]["y"] for c in range(N_CORES)], axis=0)
    return y.reshape(B, C, H, W).astype(np.float32)



# revision 10
# speedup vs baseline: 1.1598x; 1.1598x over previous
"""DCM (dynamic conv module) Trainium2 kernel, v2.

Reference computation (per sample b, channel c):
  f[b,c,3,3]  = adaptive_avg_pool2d(x[b,c], 3)        # dynamic depthwise filter
  out[b,c]    = depthwise_conv3x3(x[b,c], f[b,c])     # zero padding 1
  y           = relu(batchnorm_train(out, gamma, beta))  # batch stats over (B,H,W)

Sharding: data-parallel over batch B=16 across 8 cores (2 samples/core).
Sync-BN via a [C,2] AllReduce of per-channel (sum, sumsq).

v2 layout: x is uploaded as fp16 and DMA'd ONCE per sample into a resident
padded SBUF tile [C, 1 + W + HW + 2W] (top pad row, bottom pad rows, 1-elem
lead pad). Channels C=128 on partitions. Per 16-row output tile:
  - PE: 6 taps (dj=0 and dj=+1) as diag(f) fp16 matmuls accumulated in PSUM;
    the dj=+1 taps wrap at w=W-1 and are fixed up in the DVE tap tile.
  - DVE: 3 taps (dj=-1) with exact zero-padding via strided views, written
    into the fp16 output tile directly; plus the 3 wrap fixups.
  - Pool: merges psum + DVE-tap tile (fused per-channel sum accumulation).
  - ACT: squares the merged tile into psum-as-junk with fused sumsq accum.
Pooling for f runs as 9 region reduces (DVE) on the resident fp16 x.
After a [C,2] AllReduce (two warmup collectives absorb ncfw ramp + core
skew), BN+ReLU is applied in-place (ACT/DVE split) and DMA'd out as fp16.
"""

import ml_dtypes
import numpy as np

# ---------------------------------------------------------------- constants
B, C, H, W = 16, 128, 128, 128
N_CORES = 8
BL = B // N_CORES          # samples per core
HW = H * W                 # 16384 free elems per plane
FS = 3
BN_EPS = 1e-5

ROWS = 16                  # output rows per psum tile
NCHUNK = H // ROWS         # 8 conv tiles per plane
TILE_F = ROWS * W          # 2048 free elems per psum tile
NPSUM = NCHUNK * BL        # conv tiles per core

XOFF = 1 + W               # offset of x[0,0] in the resident tile
XR_F = 1 + W + HW + 2 * W  # lead pad, top pad row, plane, two bottom pad rows

NDMA = 4                   # x DMA chunks per sample (32 rows each)

# adaptive_avg_pool2d(3) bin boundaries (PyTorch convention)
SH = [(i * H) // FS for i in range(FS)]
EH = [-((-(i + 1) * H) // FS) for i in range(FS)]
SW = [(i * W) // FS for i in range(FS)]
EW = [-((-(i + 1) * W) // FS) for i in range(FS)]

MM_N = 512                 # psum-bank-sized moving slices
NSL = TILE_F // MM_N

# tap index t = 3*i + j with i = di+1 (rows), j = dj+1 (cols).
# PE taps run as diag(f) matmuls; the dj=+1 ones wrap at w=W-1 (fixed up in
# the DVE tile). DVE taps: center tap first (full width, overwrites), then
# the dj=-1 taps on cols 1..W-1 (exact zero padding via view clipping).
PE_TAPS = [(0, 1), (2, 1), (0, 2), (1, 2), (2, 2)]
DVE_TAPS = [(1, 1), (0, 0), (1, 0), (2, 0)]

# BN-apply engine split: True -> ACT, False -> DVE (2 ops)
BN_ON_ACT = [True, False] * 8


def _counts_recip():
    cr = np.empty((C, FS * FS), dtype=np.float32)
    for i in range(FS):
        for j in range(FS):
            cr[:, 3 * i + j] = 1.0 / float((EH[i] - SH[i]) * (EW[j] - SW[j]))
    return cr


def build_nc(n_cores: int = N_CORES):
    """Build + compile the per-core Bass program (identical on all cores)."""
    import concourse.bacc as bacc
    import concourse.tile as tile
    from concourse import mybir

    f32 = mybir.dt.float32
    f16 = mybir.dt.bfloat16
    AT = mybir.ActivationFunctionType
    OP = mybir.AluOpType
    AX = mybir.AxisListType

    ntot = float(n_cores * BL * HW)   # BN element count per channel

    nc = bacc.Bacc(
        "TRN2",
        target_bir_lowering=False,
        debug=False,
        num_devices=n_cores,
    )

    x_d = nc.dram_tensor("x", [BL, C, HW], f16, kind="ExternalInput").ap()
    gamma_d = nc.dram_tensor("gamma", [C, 1], f32, kind="ExternalInput").ap()
    beta_d = nc.dram_tensor("beta", [C, 1], f32, kind="ExternalInput").ap()
    ident_d = nc.dram_tensor("ident", [C, C], f16, kind="ExternalInput").ap()
    crecip_d = nc.dram_tensor("crecip", [C, FS * FS], f32, kind="ExternalInput").ap()
    y_d = nc.dram_tensor("y", [BL, C, HW], f16, kind="ExternalOutput").ap()

    with tile.TileContext(nc) as tc:
        with (
            tc.tile_pool(name="singles", bufs=1) as singles,
            tc.tile_pool(name="xpool", bufs=2) as xpool,
            tc.tile_pool(name="otres", bufs=NPSUM) as otres,
            tc.tile_pool(name="psum", bufs=2, space="PSUM") as psum,
            tc.tile_pool(name="fpool", bufs=2) as fpool,
            tc.tile_pool(name="scrp", bufs=2) as scrp,
            tc.tile_pool(name="diagp", bufs=2 * len(PE_TAPS)) as diagp,
            tc.tile_pool(name="statp", bufs=1) as statp,
            tc.tile_pool(name="dram", bufs=1, space="DRAM") as dram,
        ):
            # ---- constants
            gamma_s = singles.tile([C, 1], f32, tag="gamma")
            nc.sync.dma_start(out=gamma_s[:], in_=gamma_d[:, :])
            beta_s = singles.tile([C, 1], f32, tag="beta")
            nc.sync.dma_start(out=beta_s[:], in_=beta_d[:, :])
            ident_s = singles.tile([C, C], f16, tag="ident")
            nc.sync.dma_start(out=ident_s[:], in_=ident_d[:, :])
            crecip_s = singles.tile([C, FS * FS], f32, tag="crecip")
            nc.sync.dma_start(out=crecip_s[:], in_=crecip_d[:, :])

            sums = statp.tile([C, NPSUM], f32, tag="sums")
            sumsq = statp.tile([C, NPSUM], f32, tag="sumsq")

            # Warmup AllReduce #1 at kernel start: absorbs the one-time ncfw
            # ramp so the real stats AllReduce is cheaper. Runs concurrently
            # with the first pooling pass.
            warm = statp.tile([C, 2], f32, tag="warm")
            nc.gpsimd.memset(warm[:], 0.0)
            dw_in = dram.tile([C, 2], f32, tag="dw_in")
            dw_out = dram.tile([C, 2], f32, tag="dw_out")
            nc.sync.dma_start(out=dw_in[:], in_=warm[:])
            nc.gpsimd.collective_compute(
                "AllReduce",
                OP.add,
                replica_groups=[list(range(n_cores))],
                ins=[dw_in[:].opt()],
                outs=[dw_out[:].opt()],
            )

            out_tiles = []
            kpt = 0
            warm2_issued = False

            for s in range(BL):
                # ---------------- load sample s into resident padded tile
                xr = xpool.tile([C, XR_F], f16, tag="xr")
                nc.vector.memset(xr[:, 0:XOFF], 0.0)
                nc.vector.memset(xr[:, XOFF + HW:XR_F], 0.0)
                rows_per = H // NDMA
                for d in range(NDMA):
                    lo = d * rows_per * W
                    hi = (d + 1) * rows_per * W
                    nc.sync.dma_start(
                        out=xr[:, XOFF + lo:XOFF + hi], in_=x_d[s, :, lo:hi]
                    )

                def xrows(r0, nrows, xr=xr):
                    """[C, nrows, W] view of x rows r0 .. r0+nrows (pads ok)."""
                    start = XOFF + r0 * W
                    return xr[:, start:start + nrows * W].rearrange(
                        "p (r w) -> p r w", w=W
                    )

                # ---------------- adaptive pool: 9 region reduces -> fsum
                fsum = fpool.tile([C, FS * FS], f32, tag="fsum")
                for i in range(FS):
                    for j in range(FS):
                        t = 3 * i + j
                        reg = xrows(SH[i], EH[i] - SH[i])[:, :, SW[j]:EW[j]]
                        nc.vector.tensor_reduce(
                            out=fsum[:, t:t + 1], in_=reg, axis=AX.XY, op=OP.add
                        )
                fT = fpool.tile([C, FS * FS], f32, tag="fT")
                nc.vector.tensor_mul(fT[:], fsum[:], crecip_s[:])
                fneg = fpool.tile([C, FS * FS], f32, tag="fneg")
                nc.vector.tensor_scalar_mul(fneg[:], fT[:], -1.0)
                diags = {}
                for (i, j) in PE_TAPS:
                    t = 3 * i + j
                    dg = diagp.tile([C, C], f16, tag="diag")
                    nc.vector.tensor_scalar_mul(dg[:], ident_s[:], fT[:, t:t + 1])
                    diags[t] = dg

                # ---------------- conv tiles
                for c in range(NCHUNK):
                    r0 = c * ROWS

                    # DVE taps into the fp16 out tile: center tap overwrites
                    # the full width, then dj=-1 taps on cols 1..W-1 (exact
                    # zero padding), then the wrap fixups for the PE dj=+1
                    # taps: out[h, W-1] wrongly gets f[i,2] * x[h+di, W]
                    # == x[h+di+1, 0]; pre-subtract it here.
                    ot = otres.tile([C, TILE_F], f16, tag="ot")
                    otv = ot[:].rearrange("p (r w) -> p r w", w=W)
                    for k, (i, j) in enumerate(DVE_TAPS):
                        di = i - 1
                        t = 3 * i + j
                        if j == 1:  # center tap, full width
                            dst, src = otv[:, :, :], xrows(r0 + di, ROWS)
                        else:       # dj = -1
                            dst = otv[:, :, 1:W]
                            src = xrows(r0 + di, ROWS)[:, :, 0:W - 1]
                        if k == 0:
                            nc.vector.tensor_scalar_mul(
                                dst, src, fT[:, t:t + 1]
                            )
                        else:
                            nc.vector.scalar_tensor_tensor(
                                out=dst, in0=src, scalar=fT[:, t:t + 1],
                                in1=dst, op0=OP.mult, op1=OP.add,
                            )
                    for i in range(FS):
                        di = i - 1
                        src = xrows(r0 + di + 1, ROWS)[:, :, 0:1]
                        nc.vector.scalar_tensor_tensor(
                            out=otv[:, :, W - 1:W],
                            in0=src,
                            scalar=fneg[:, 3 * i + 2:3 * i + 3],
                            in1=otv[:, :, W - 1:W],
                            op0=OP.mult,
                            op1=OP.add,
                        )

                    # PE taps + identity-matmul merge of the DVE tile, with
                    # sequential per-bank accumulation groups (slice-outer).
                    pt = psum.tile([C, TILE_F], f32, tag="pt")
                    for sl in range(NSL):
                        for ti, (i, j) in enumerate(PE_TAPS):
                            di, dj = i - 1, j - 1
                            mbase = XOFF + (r0 + di) * W + dj
                            nc.tensor.matmul(
                                pt[:, sl * MM_N:(sl + 1) * MM_N],
                                diags[3 * i + j][:],
                                xr[:, mbase + sl * MM_N:mbase + (sl + 1) * MM_N],
                                start=(ti == 0),
                                stop=False,
                            )
                        nc.tensor.matmul(
                            pt[:, sl * MM_N:(sl + 1) * MM_N],
                            ident_s[:],
                            ot[:, sl * MM_N:(sl + 1) * MM_N],
                            start=False,
                            stop=True,
                        )

                    # ACT: drain psum over the tap tile (+ per-channel sum)
                    nc.scalar.activation(
                        out=ot[:], in_=pt[:], func=AT.Copy,
                        accum_out=sums[:, kpt:kpt + 1],
                    )
                    # ACT: sum of squares of the merged tile (psum as junk)
                    nc.scalar.activation(
                        out=pt[:], in_=ot[:], func=AT.Square,
                        accum_out=sumsq[:, kpt:kpt + 1],
                    )
                    out_tiles.append((s, c, ot))
                    kpt += 1

            # ---------------- sync-BN stats AllReduce
            arin = statp.tile([C, 2], f32, tag="arin")
            nc.vector.tensor_reduce(out=arin[:, 0:1], in_=sums[:], axis=AX.X, op=OP.add)
            nc.vector.tensor_reduce(out=arin[:, 1:2], in_=sumsq[:], axis=AX.X, op=OP.add)
            d_in = dram.tile([C, 2], f32, tag="d_in")
            d_out = dram.tile([C, 2], f32, tag="d_out")
            nc.sync.dma_start(out=d_in[:], in_=arin[:])
            nc.gpsimd.collective_compute(
                "AllReduce",
                OP.add,
                replica_groups=[list(range(n_cores))],
                ins=[d_in[:].opt()],
                outs=[d_out[:].opt()],
            )
            aro = statp.tile([C, 2], f32, tag="aro")
            nc.sync.dma_start(out=aro[:], in_=d_out[:])

            # ---------------- BN scale/shift (all [C,1], fp32)
            mean = statp.tile([C, 1], f32, tag="mean")
            nc.vector.tensor_scalar_mul(mean[:], aro[:, 0:1], 1.0 / ntot)
            ex2 = statp.tile([C, 1], f32, tag="ex2")
            nc.vector.tensor_scalar_mul(ex2[:], aro[:, 1:2], 1.0 / ntot)
            var = statp.tile([C, 1], f32, tag="var")
            nc.vector.tensor_mul(var[:], mean[:], mean[:])
            nc.vector.tensor_sub(var[:], ex2[:], var[:])
            veps = statp.tile([C, 1], f32, tag="veps")
            nc.vector.tensor_scalar_add(veps[:], var[:], BN_EPS)
            eps_t = statp.tile([C, 1], f32, tag="eps_t")
            nc.vector.memset(eps_t[:], BN_EPS)
            sd = statp.tile([C, 1], f32, tag="sd")
            nc.scalar.activation(out=sd[:], in_=var[:], func=AT.Sqrt, bias=eps_t[:])
            z = statp.tile([C, 1], f32, tag="z")
            nc.vector.reciprocal(z[:], sd[:])
            # one Newton step: z <- z * (1.5 - 0.5 * veps * z^2)
            nt = statp.tile([C, 1], f32, tag="nt")
            nc.vector.tensor_mul(nt[:], z[:], z[:])
            nc.vector.tensor_mul(nt[:], nt[:], veps[:])
            nc.vector.tensor_scalar(
                out=nt[:], in0=nt[:], scalar1=-0.5, scalar2=1.5,
                op0=OP.mult, op1=OP.add,
            )
            nc.vector.tensor_mul(z[:], z[:], nt[:])
            scale_t = statp.tile([C, 1], f32, tag="scale_t")
            nc.vector.tensor_mul(scale_t[:], gamma_s[:], z[:])
            shift_t = statp.tile([C, 1], f32, tag="shift_t")
            nc.vector.tensor_mul(shift_t[:], mean[:], scale_t[:])
            nc.vector.tensor_sub(shift_t[:], beta_s[:], shift_t[:])

            # ---------------- BN apply + ReLU + writeback (ACT / DVE split)
            for idx, (s, c, ot) in enumerate(out_tiles):
                if BN_ON_ACT[idx]:
                    nc.scalar.activation(
                        out=ot[:], in_=ot[:], func=AT.Relu,
                        scale=scale_t[:], bias=shift_t[:],
                    )
                else:
                    nc.vector.tensor_scalar(
                        out=ot[:], in0=ot[:],
                        scalar1=scale_t[:], scalar2=shift_t[:],
                        op0=OP.mult, op1=OP.add,
                    )
                    nc.vector.tensor_scalar_max(ot[:], ot[:], 0.0)
                nc.sync.dma_start(
                    out=y_d[s, :, c * TILE_F:(c + 1) * TILE_F], in_=ot[:],
                )

    nc.compile()
    return nc


_NC_CACHE = {}


def _get_nc(n_cores: int = N_CORES):
    if n_cores not in _NC_CACHE:
        _NC_CACHE[n_cores] = build_nc(n_cores)
    return _NC_CACHE[n_cores]


def make_in_maps(x: np.ndarray, gamma: np.ndarray, beta: np.ndarray,
                 n_cores: int = N_CORES):
    x_r = np.ascontiguousarray(
        np.asarray(x, dtype=np.float32).reshape(B, C, HW).astype(ml_dtypes.bfloat16)
    )
    g = np.ascontiguousarray(np.asarray(gamma, dtype=np.float32).reshape(C, 1))
    b = np.ascontiguousarray(np.asarray(beta, dtype=np.float32).reshape(C, 1))
    ident = np.eye(C, dtype=ml_dtypes.bfloat16)
    crecip = _counts_recip()
    maps = []
    for core in range(n_cores):
        maps.append({
            "x": x_r[core * BL:(core + 1) * BL],
            "gamma": g,
            "beta": b,
            "ident": ident,
            "crecip": crecip,
        })
    return maps


def kernel(x, gamma, beta):
    from concourse import bass_utils

    nc = _get_nc(N_CORES)
    in_maps = make_in_maps(x, gamma, beta, N_CORES)
    res = bass_utils.run_bass_kernel_spmd(nc, in_maps, core_ids=list(range(N_CORES)))
    y = np.concatenate([res.results[c]["y"] for c in range(N_CORES)], axis=0)
    return y.reshape(B, C, H, W).astype(np.float32)


# revision 16
# speedup vs baseline: 1.3066x; 1.1266x over previous
"""DCM (dynamic conv module) Trainium2 kernel, v2.

Reference computation (per sample b, channel c):
  f[b,c,3,3]  = adaptive_avg_pool2d(x[b,c], 3)        # dynamic depthwise filter
  out[b,c]    = depthwise_conv3x3(x[b,c], f[b,c])     # zero padding 1
  y           = relu(batchnorm_train(out, gamma, beta))  # batch stats over (B,H,W)

Sharding: data-parallel over batch B=16 across 8 cores (2 samples/core).
Sync-BN via a [C,2] AllReduce of per-channel (sum, sumsq).

v2 layout: x is uploaded as fp16 and DMA'd ONCE per sample into a resident
padded SBUF tile [C, 1 + W + HW + 2W] (top pad row, bottom pad rows, 1-elem
lead pad). Channels C=128 on partitions. Per 16-row output tile:
  - PE: 6 taps (dj=0 and dj=+1) as diag(f) fp16 matmuls accumulated in PSUM;
    the dj=+1 taps wrap at w=W-1 and are fixed up in the DVE tap tile.
  - DVE: 3 taps (dj=-1) with exact zero-padding via strided views, written
    into the fp16 output tile directly; plus the 3 wrap fixups.
  - Pool: merges psum + DVE-tap tile (fused per-channel sum accumulation).
  - ACT: squares the merged tile into psum-as-junk with fused sumsq accum.
Pooling for f runs as 9 region reduces (DVE) on the resident fp16 x.
After a [C,2] AllReduce (two warmup collectives absorb ncfw ramp + core
skew), BN+ReLU is applied in-place (ACT/DVE split) and DMA'd out as fp16.
"""

import ml_dtypes
import numpy as np

# ---------------------------------------------------------------- constants
B, C, H, W = 16, 128, 128, 128
N_CORES = 8
BL = B // N_CORES          # samples per core
HW = H * W                 # 16384 free elems per plane
FS = 3
BN_EPS = 1e-5

ROWS = 16                  # output rows per psum tile
NCHUNK = H // ROWS         # 8 conv tiles per plane
TILE_F = ROWS * W          # 2048 free elems per psum tile
NPSUM = NCHUNK * BL        # conv tiles per core

XOFF = 1 + 2 * W           # offset of x[0,0] in the resident tile
XR_F = 1 + 2 * W + HW + 2 * W  # lead pad, 2 top pad rows, plane, 2 bottom pad rows

NDMA = 4                   # x DMA chunks per sample (32 rows each)

# adaptive_avg_pool2d(3) bin boundaries (PyTorch convention)
SH = [(i * H) // FS for i in range(FS)]
EH = [-((-(i + 1) * H) // FS) for i in range(FS)]
SW = [(i * W) // FS for i in range(FS)]
EW = [-((-(i + 1) * W) // FS) for i in range(FS)]

MM_N = 512                 # psum-bank-sized moving slices
NSL = TILE_F // MM_N

# tap index t = 3*i + j with i = di+1 (rows), j = dj+1 (cols).
# PE taps run as diag(f) matmuls into PSUM. dj=+1 taps wrap at w=W-1 and
# dj=-1 taps wrap at w=0; both are fixed up on the drained tile. After the
# ACT drain, the two remaining taps accumulate into the drained tile on
# DVE; the full-width center tap runs last so its accum_out yields the
# complete per-channel tile sum.
PE_TAPS = [(0, 1), (2, 1), (0, 2), (1, 2), (2, 2), (0, 0), (1, 0)]
DVE_TAPS = [(2, 0), (1, 1)]

# engine for each pooling region (i, j): 'v' = DVE, 'a' = ACT
POOL_REGION_ENG = {
    (0, 0): 'v', (0, 1): 'a', (0, 2): 'v',
    (1, 0): 'a', (1, 1): 'v', (1, 2): 'a',
    (2, 0): 'v', (2, 1): 'a', (2, 2): 'a',
}

# BN-apply engine split: True -> ACT, False -> DVE (2 ops)
BN_ON_ACT = ([True, True, False] * 6)[:16]


def _counts_recip():
    cr = np.empty((C, FS * FS), dtype=np.float32)
    for i in range(FS):
        for j in range(FS):
            cr[:, 3 * i + j] = 1.0 / float((EH[i] - SH[i]) * (EW[j] - SW[j]))
    return cr


def build_nc(n_cores: int = N_CORES):
    """Build + compile the per-core Bass program (identical on all cores)."""
    import concourse.bacc as bacc
    import concourse.tile as tile
    from concourse import mybir

    f32 = mybir.dt.float32
    f16 = mybir.dt.bfloat16
    AT = mybir.ActivationFunctionType
    OP = mybir.AluOpType
    AX = mybir.AxisListType

    ntot = float(n_cores * BL * HW)   # BN element count per channel

    nc = bacc.Bacc(
        "TRN2",
        target_bir_lowering=False,
        debug=False,
        num_devices=n_cores,
    )

    x_d = nc.dram_tensor("x", [BL, C, HW], f16, kind="ExternalInput").ap()
    gamma_d = nc.dram_tensor("gamma", [C, 1], f32, kind="ExternalInput").ap()
    beta_d = nc.dram_tensor("beta", [C, 1], f32, kind="ExternalInput").ap()
    ident_d = nc.dram_tensor("ident", [C, C], f16, kind="ExternalInput").ap()
    crecip_d = nc.dram_tensor("crecip", [C, FS * FS], f32, kind="ExternalInput").ap()
    y_d = nc.dram_tensor("y", [BL, C, HW], f16, kind="ExternalOutput").ap()

    with tile.TileContext(nc) as tc:
        with (
            tc.tile_pool(name="singles", bufs=1) as singles,
            tc.tile_pool(name="xpool", bufs=2) as xpool,
            tc.tile_pool(name="otres", bufs=NPSUM) as otres,
            tc.tile_pool(name="psum", bufs=2, space="PSUM") as psum,
            tc.tile_pool(name="fpool", bufs=2) as fpool,
            tc.tile_pool(name="scrp", bufs=2) as scrp,
            tc.tile_pool(name="diagp", bufs=2 * len(PE_TAPS)) as diagp,
            tc.tile_pool(name="statp", bufs=1) as statp,
            tc.tile_pool(name="dram", bufs=1, space="DRAM") as dram,
        ):
            # ---- x DMAs first: sample 0's data is the critical path
            xr_tiles = []
            rows_per = H // NDMA
            for s in range(BL):
                xr = xpool.tile([C, XR_F], f16, tag="xr")
                nc.vector.memset(xr[:, 0:XOFF], 0.0)
                nc.vector.memset(xr[:, XOFF + HW:XR_F], 0.0)
                for d in range(NDMA):
                    lo = d * rows_per * W
                    hi = (d + 1) * rows_per * W
                    nc.sync.dma_start(
                        out=xr[:, XOFF + lo:XOFF + hi], in_=x_d[s, :, lo:hi]
                    )
                xr_tiles.append(xr)

            # ---- constants
            gamma_s = singles.tile([C, 1], f32, tag="gamma")
            nc.sync.dma_start(out=gamma_s[:], in_=gamma_d[:, :])
            beta_s = singles.tile([C, 1], f32, tag="beta")
            nc.sync.dma_start(out=beta_s[:], in_=beta_d[:, :])
            ident_s = singles.tile([C, C], f16, tag="ident")
            nc.sync.dma_start(out=ident_s[:], in_=ident_d[:, :])
            crecip_s = singles.tile([C, FS * FS], f32, tag="crecip")
            nc.sync.dma_start(out=crecip_s[:], in_=crecip_d[:, :])

            sums = statp.tile([C, NPSUM], f32, tag="sums")
            sumsq = statp.tile([C, NPSUM], f32, tag="sumsq")

            # Warmup AllReduce #1 at kernel start: absorbs the one-time ncfw
            # ramp so the real stats AllReduce is cheaper. Runs concurrently
            # with the first pooling pass.
            warm = statp.tile([C, 2], f32, tag="warm")
            nc.gpsimd.memset(warm[:], 0.0)
            dw_in = dram.tile([C, 2], f32, tag="dw_in")
            dw_out = dram.tile([C, 2], f32, tag="dw_out")
            nc.sync.dma_start(out=dw_in[:], in_=warm[:])
            nc.gpsimd.collective_compute(
                "AllReduce",
                OP.add,
                replica_groups=[list(range(n_cores))],
                ins=[dw_in[:].opt()],
                outs=[dw_out[:].opt()],
            )

            out_tiles = []
            kpt = 0

            for s in range(BL):
                xr = xr_tiles[s]

                def xrows(r0, nrows, xr=xr):
                    """[C, nrows, W] view of x rows r0 .. r0+nrows (pads ok)."""
                    start = XOFF + r0 * W
                    return xr[:, start:start + nrows * W].rearrange(
                        "p (r w) -> p r w", w=W
                    )

                # ---------------- adaptive pool: 9 region reduces -> fsum,
                # split DVE / ACT (ACT uses Copy + accum into a junk tile)
                fsum = fpool.tile([C, FS * FS], f32, tag="fsum")
                for i in range(FS):
                    for j in range(FS):
                        t = 3 * i + j
                        nr, nw = EH[i] - SH[i], EW[j] - SW[j]
                        reg = xrows(SH[i], nr)[:, :, SW[j]:EW[j]]
                        if POOL_REGION_ENG[(i, j)] == 'v':
                            nc.vector.tensor_reduce(
                                out=fsum[:, t:t + 1], in_=reg,
                                axis=AX.XY, op=OP.add,
                            )
                        else:
                            junk = scrp.tile([C, TILE_F], f16, tag="scr")
                            jv = junk[:, 0:nr * nw].rearrange(
                                "p (r w) -> p r w", w=nw
                            )
                            nc.scalar.activation(
                                out=jv, in_=reg, func=AT.Copy,
                                accum_out=fsum[:, t:t + 1],
                            )
                fT = fpool.tile([C, FS * FS], f32, tag="fT")
                nc.vector.tensor_mul(fT[:], fsum[:], crecip_s[:])
                fneg = fpool.tile([C, FS * FS], f32, tag="fneg")
                nc.vector.tensor_scalar_mul(fneg[:], fT[:], -1.0)
                diags = {}
                for (i, j) in PE_TAPS:
                    t = 3 * i + j
                    dg = diagp.tile([C, C], f16, tag="diag")
                    nc.vector.tensor_scalar_mul(dg[:], ident_s[:], fT[:, t:t + 1])
                    diags[t] = dg

                # ---------------- conv tiles
                for c in range(NCHUNK):
                    r0 = c * ROWS

                    # PE taps into PSUM (slice-outer accumulation groups)
                    pt = psum.tile([C, TILE_F], f32, tag="pt")
                    for sl in range(NSL):
                        for ti, (i, j) in enumerate(PE_TAPS):
                            di, dj = i - 1, j - 1
                            mbase = XOFF + (r0 + di) * W + dj
                            nc.tensor.matmul(
                                pt[:, sl * MM_N:(sl + 1) * MM_N],
                                diags[3 * i + j][:],
                                xr[:, mbase + sl * MM_N:mbase + (sl + 1) * MM_N],
                                start=(ti == 0),
                                stop=(ti == len(PE_TAPS) - 1),
                            )

                    # ACT: drain psum to the bf16 out tile (frees psum early;
                    # PE never waits on the vector engines)
                    ot = otres.tile([C, TILE_F], f16, tag="ot")
                    nc.scalar.activation(out=ot[:], in_=pt[:], func=AT.Copy)
                    otv = ot[:].rearrange("p (r w) -> p r w", w=W)

                    # Wrap fixups on the drained tile. PE dj=+1 taps:
                    # out[h, W-1] wrongly got f[i,2] * x[h+di, W] ==
                    # x[h+di+1, 0]. PE dj=-1 taps: out[h, 0] wrongly got
                    # f[i,0] * x[h+di, -1] == x[h+di-1, W-1]. Subtract both.
                    for (i, j) in PE_TAPS:
                        if j == 1:
                            continue
                        di = i - 1
                        t = 3 * i + j
                        if j == 2:
                            dst = otv[:, :, W - 1:W]
                            src = xrows(r0 + di + 1, ROWS)[:, :, 0:1]
                        else:
                            dst = otv[:, :, 0:1]
                            src = xrows(r0 + di - 1, ROWS)[:, :, W - 1:W]
                        nc.vector.scalar_tensor_tensor(
                            out=dst, in0=src,
                            scalar=fneg[:, t:t + 1],
                            in1=dst, op0=OP.mult, op1=OP.add,
                        )
                    # DVE taps: (2,0) on cols 1..W-1 (exact zero padding),
                    # then the full-width center tap with the tile-sum accum.
                    nc.vector.scalar_tensor_tensor(
                        out=otv[:, :, 1:W],
                        in0=xrows(r0 + 1, ROWS)[:, :, 0:W - 1],
                        scalar=fT[:, 6:7],
                        in1=otv[:, :, 1:W],
                        op0=OP.mult, op1=OP.add,
                    )
                    nc.vector.scalar_tensor_tensor(
                        out=otv[:, :, :],
                        in0=xrows(r0, ROWS),
                        scalar=fT[:, 4:5],
                        in1=otv[:, :, :],
                        op0=OP.mult, op1=OP.add,
                        accum_out=sums[:, kpt:kpt + 1],
                    )
                    # ACT: sum of squares of the completed tile
                    scr = scrp.tile([C, TILE_F], f16, tag="scr")
                    nc.scalar.activation(
                        out=scr[:], in_=ot[:], func=AT.Square,
                        accum_out=sumsq[:, kpt:kpt + 1],
                    )
                    out_tiles.append((s, c, ot))
                    kpt += 1

            # ---------------- sync-BN stats AllReduce
            arin = statp.tile([C, 2], f32, tag="arin")
            nc.vector.tensor_reduce(out=arin[:, 0:1], in_=sums[:], axis=AX.X, op=OP.add)
            nc.vector.tensor_reduce(out=arin[:, 1:2], in_=sumsq[:], axis=AX.X, op=OP.add)
            d_in = dram.tile([C, 2], f32, tag="d_in")
            d_out = dram.tile([C, 2], f32, tag="d_out")
            nc.sync.dma_start(out=d_in[:], in_=arin[:])
            nc.gpsimd.collective_compute(
                "AllReduce",
                OP.add,
                replica_groups=[list(range(n_cores))],
                ins=[d_in[:].opt()],
                outs=[d_out[:].opt()],
            )
            aro = statp.tile([C, 2], f32, tag="aro")
            nc.sync.dma_start(out=aro[:], in_=d_out[:])

            # ---------------- BN scale/shift (all [C,1], fp32)
            mean = statp.tile([C, 1], f32, tag="mean")
            nc.vector.tensor_scalar_mul(mean[:], aro[:, 0:1], 1.0 / ntot)
            ex2 = statp.tile([C, 1], f32, tag="ex2")
            nc.vector.tensor_scalar_mul(ex2[:], aro[:, 1:2], 1.0 / ntot)
            var = statp.tile([C, 1], f32, tag="var")
            nc.vector.tensor_mul(var[:], mean[:], mean[:])
            nc.vector.tensor_sub(var[:], ex2[:], var[:])
            veps = statp.tile([C, 1], f32, tag="veps")
            nc.vector.tensor_scalar_add(veps[:], var[:], BN_EPS)
            eps_t = statp.tile([C, 1], f32, tag="eps_t")
            nc.vector.memset(eps_t[:], BN_EPS)
            sd = statp.tile([C, 1], f32, tag="sd")
            nc.scalar.activation(out=sd[:], in_=var[:], func=AT.Sqrt, bias=eps_t[:])
            z = statp.tile([C, 1], f32, tag="z")
            nc.vector.reciprocal(z[:], sd[:])
            # one Newton step: z <- z * (1.5 - 0.5 * veps * z^2)
            nt = statp.tile([C, 1], f32, tag="nt")
            nc.vector.tensor_mul(nt[:], z[:], z[:])
            nc.vector.tensor_mul(nt[:], nt[:], veps[:])
            nc.vector.tensor_scalar(
                out=nt[:], in0=nt[:], scalar1=-0.5, scalar2=1.5,
                op0=OP.mult, op1=OP.add,
            )
            nc.vector.tensor_mul(z[:], z[:], nt[:])
            scale_t = statp.tile([C, 1], f32, tag="scale_t")
            nc.vector.tensor_mul(scale_t[:], gamma_s[:], z[:])
            shift_t = statp.tile([C, 1], f32, tag="shift_t")
            nc.vector.tensor_mul(shift_t[:], mean[:], scale_t[:])
            nc.vector.tensor_sub(shift_t[:], beta_s[:], shift_t[:])

            # ---------------- BN apply + ReLU + writeback (ACT / DVE split)
            for idx, (s, c, ot) in enumerate(out_tiles):
                if BN_ON_ACT[idx]:
                    nc.scalar.activation(
                        out=ot[:], in_=ot[:], func=AT.Relu,
                        scale=scale_t[:], bias=shift_t[:],
                    )
                else:
                    nc.vector.tensor_scalar(
                        out=ot[:], in0=ot[:],
                        scalar1=scale_t[:], scalar2=shift_t[:],
                        op0=OP.mult, op1=OP.add,
                    )
                    nc.vector.tensor_scalar_max(ot[:], ot[:], 0.0)
                nc.sync.dma_start(
                    out=y_d[s, :, c * TILE_F:(c + 1) * TILE_F], in_=ot[:],
                )

    nc.compile()
    return nc


_NC_CACHE = {}


def _get_nc(n_cores: int = N_CORES):
    if n_cores not in _NC_CACHE:
        _NC_CACHE[n_cores] = build_nc(n_cores)
    return _NC_CACHE[n_cores]


def make_in_maps(x: np.ndarray, gamma: np.ndarray, beta: np.ndarray,
                 n_cores: int = N_CORES):
    x_r = np.ascontiguousarray(
        np.asarray(x, dtype=np.float32).reshape(B, C, HW).astype(ml_dtypes.bfloat16)
    )
    g = np.ascontiguousarray(np.asarray(gamma, dtype=np.float32).reshape(C, 1))
    b = np.ascontiguousarray(np.asarray(beta, dtype=np.float32).reshape(C, 1))
    ident = np.eye(C, dtype=ml_dtypes.bfloat16)
    crecip = _counts_recip()
    maps = []
    for core in range(n_cores):
        maps.append({
            "x": x_r[core * BL:(core + 1) * BL],
            "gamma": g,
            "beta": b,
            "ident": ident,
            "crecip": crecip,
        })
    return maps


def kernel(x, gamma, beta):
    from concourse import bass_utils

    nc = _get_nc(N_CORES)
    in_maps = make_in_maps(x, gamma, beta, N_CORES)
    res = bass_utils.run_bass_kernel_spmd(nc, in_maps, core_ids=list(range(N_CORES)))
    y = np.concatenate([res.results[c]["y"] for c in range(N_CORES)], axis=0)
    return y.reshape(B, C, H, W).astype(np.float32)
